# revision 27
# baseline (speedup 1.0000x reference)
"""Trainium2 Bass kernel for nn_EqvLBAFeedForward (gnn_message_passing).

Reference computation (per sample z):
  r[a,b]   = |xyz[a]-xyz[b]|                                  [N,N]
  basis_k  = exp(-0.3*(r-c_k)^2), c = [0,5,10]                [N,N,3]
  hid      = swish(basis @ rw1)                               [N,N,H]
  K        = hid @ rw2  -> [N,N,C,C]
  out[a,i] = sum_{b,j} K[a,b,i,j] x[b,j] / sqrt(N)            [N,C]
  pooled   = sum_a mask[a]*|out[a,:]| ; normalize ; MLP head  -> scalar

Key algebraic restructuring: never materialize K. Define
  W2x[b,h,i] = sum_j rw2[h, i*C+j] * x[b,j]
then
  out[a,i] = sum_{b,h} hid[a,b,h] * W2x[b,h,i]
which drops the dominant contraction from O(N^2 H C^2) to O(N^2 H C).

Sharding: 8 cores = (z in 0..3) x (half of the mask-kept output points a).
Masked-out points are compacted away on the host (they only feed the pool).

Device pipeline per core (z fixed, A padded kept-half points):
  step3: pre_hid[h', (b,a)] via K=3 matmuls, rw1 stationary in 4 PE
         row-strips (partitions 32s+k), V3 basis laid out [3, pairs].
  swish: ACT Silu on each 3-bank PSUM tile -> hid in SBUF.
  step5: per b, K=128 matmul lhsT=W2x[:,32b:32b+32] (h' padded to 128),
         rhs=hid slice, accumulated into a column-tiled PSUM tile
         (strip j = b%4 at partitions 32j..32j+32).
Host folds the 4 column strips, takes |.|, pools, normalizes, runs the
tiny MLP head.
"""

import os
import numpy as np

MAX_RADIUS = 10.0
NUM_BASIS = 3
H = 100
C = 32
N = 256
B = 4
N_CORES = 8
GAMMA = NUM_BASIS / MAX_RADIUS  # 1/spacing = 0.3
CENTERS = np.linspace(0.0, MAX_RADIUS, NUM_BASIS, dtype=np.float32)  # [0,5,10]
LEAKY_SLOPE = 0.01

LAST_RESULT = None  # BassKernelResults of the most recent device run (for test.py)

_PROGRAM_CACHE = {}


def _build_program(A):
    """Build (and cache) the Bass/Tile program for padded half-size A."""
    if A in _PROGRAM_CACHE:
        return _PROGRAM_CACHE[A]

    import concourse.bass as bass
    import concourse.tile as tile
    from concourse import mybir

    f32 = mybir.dt.float32
    NCH = (64 * A) // 512          # 512-col chunks per strip
    n_g = 4 * NCH                  # total chunks (strip-minor order: g -> (s=g%4, c=g//4))
    n_tiles = (n_g + 2) // 3       # psum3 tiles of 3 banks each

    nc = bass.Bass()
    # vcon rows 3s+k: cols 0:128 = rw1 strip weights, cols 128: = basis V3 strip
    vcon_d = nc.dram_tensor("vcon", [12, 128 + 64 * A], f32, kind="ExternalInput")
    w2sb_d = nc.dram_tensor("w2sb", [128, N * C], f32, kind="ExternalInput")
    outp_d = nc.dram_tensor("outp", [128, A], f32, kind="ExternalOutput")

    # hid column lookup: chunk g covers V3-strip cols [512*(g//4), +512) of strip g%4.
    # hid tile t = g//3, slot g%3 (cols 512*(g%3)+u mirrors chunk col u).
    def hid_pieces(b):
        """Return list of (tile_idx, col0, v_off, length) covering pair-cols of b."""
        s, v0 = b // 64, (b % 64) * A
        pieces = []
        v = v0
        while v < v0 + A:
            c = v // 512
            take = min(v0 + A - v, 512 - (v % 512))
            g = 4 * c + s
            pieces.append((g // 3, 512 * (g % 3) + (v % 512), v - v0, take))
            v += take
        return pieces

    with tile.TileContext(nc) as tc:
        with (
            tc.tile_pool(name="singles", bufs=1) as singles,
            tc.tile_pool(name="hidp", bufs=1) as hidp,
            tc.tile_pool(name="ps3", bufs=2, space=bass.MemorySpace.PSUM) as ps3,
            tc.tile_pool(name="ps5", bufs=1, space=bass.MemorySpace.PSUM) as ps5,
            tc.tile_pool(name="scr", bufs=1, space=bass.MemorySpace.PSUM) as scr,
        ):
            vcon = singles.tile([128, 128 + 64 * A], f32)
            for s in range(4):
                nc.sync.dma_start(
                    out=vcon[32 * s : 32 * s + 3, :],
                    in_=vcon_d[3 * s : 3 * s + 3, :],
                )
            rw1p = vcon[:, 0:128]
            v3 = vcon[:, 128:]
            # 3 w2 pieces: with the 4 vcon DMAs that's 7 input DMAs, so the
            # final output DMA lands on a fresh DMAHW lane (each DMA carries
            # at most one sync wait in the ISA).
            W2CUTS = [0, 32 * C, 128 * C, N * C]
            w2 = singles.tile([128, N * C], f32)
            for p in range(len(W2CUTS) - 1):
                sl = slice(W2CUTS[p], W2CUTS[p + 1])
                nc.sync.dma_start(out=w2[:, sl], in_=w2sb_d[:, sl])

            acc = ps5.tile([128, A], f32)
            scratch = scr.tile([1, 1], f32)
            w2_ready = [False] * (len(W2CUTS) - 1)

            def touch_w2(b):
                """Absorb the w2-piece DMA wait into a dummy PE matmul so real
                matmuls never carry more than one sync wait (S3_LW limit)."""
                p = next(i for i in range(len(W2CUTS) - 1) if C * b < W2CUTS[i + 1])
                if not w2_ready[p]:
                    col = W2CUTS[p]
                    nc.tensor.matmul(
                        scratch[0:1, 0:1],
                        w2[:, col : col + 1],
                        w2[:, col : col + 1],
                        start=True,
                        stop=True,
                    )
                    w2_ready[p] = True

            from concourse.tile import add_dep_helper

            hid_tiles = []
            hid_readers = {}  # tile idx -> first step-5 matmul inst reading it
            pending = list(range(N))
            strips_started = [False] * 4
            strip_count = [0] * 4
            for t in range(n_tiles):
                gs = [g for g in (3 * t, 3 * t + 1, 3 * t + 2) if g < n_g]
                pt = ps3.tile([128, 512 * 3], f32, tag="ps3t")
                for j, g in enumerate(gs):
                    s, c = g % 4, g // 4
                    mm = nc.tensor.matmul(
                        pt[:, 512 * j : 512 * (j + 1)],
                        rw1p[32 * s : 32 * s + 3, :],
                        v3[32 * s : 32 * s + 3, 512 * c : 512 * (c + 1)],
                        start=True,
                        stop=True,
                        tile_position=(32 * s, 0),
                    )
                    # schedule the slot-reusing matmul after a consumer of the
                    # swish that frees the slot, so its WAR wait is already in
                    # PE's vector clock (ISA: one sync wait per matmul)
                    if j == 0 and t - 2 in hid_readers:
                        add_dep_helper(
                            hid_readers[t - 2].ins,
                            mm.ins,
                            sync=False,
                            reason="psum3 reuse after old hid consumed",
                        )
                ht = hidp.tile([128, 512 * 3], f32, tag=f"hid{t}")
                nvalid = 512 * len(gs)
                nc.scalar.activation(
                    out=ht[:, :nvalid],
                    in_=pt[:, :nvalid],
                    func=mybir.ActivationFunctionType.Silu,
                )
                hid_tiles.append(ht)

                # emit step-5 matmuls for every b fully covered by swished chunks
                done_g = 3 * t + len(gs)
                for b in list(pending):
                    pieces = hid_pieces(b)
                    # b is ready iff all its pieces lie in finished chunks
                    if any((ti * 3 + co // 512) >= done_g for (ti, co, vo, ln) in pieces):
                        continue
                    jj = b % 4
                    strip_count[jj] += 1
                    touch_w2(b)
                    for (ti, co, vo, ln) in pieces:
                        mm5 = nc.tensor.matmul(
                            acc[32 * jj : 32 * (jj + 1), vo : vo + ln],
                            w2[:, C * b : C * (b + 1)],
                            hid_tiles[ti][:, co : co + ln],
                            start=not strips_started[jj],
                            stop=(strip_count[jj] == N // 4),
                            skip_group_check=True,
                            tile_position=(0, 32 * jj),
                        )
                        hid_readers.setdefault(ti, mm5)
                    strips_started[jj] = True
                    pending.remove(b)

            out_s = singles.tile([128, A], f32)
            nc.vector.tensor_copy(out=out_s[:], in_=acc[:])
            nc.sync.dma_start(out=outp_d[:], in_=out_s[:])

    nc.finalize()

    # The ISA allows one sync-wait per matmul (walrus puts them on the
    # LDWEIGHTS slot). A matmul reusing a psum pool slot picks up both a
    # WAR wait (ACT: the swish that read the old tile) and a same-engine
    # PE WAW wait. The PE wait is redundant — the PE issues in order and
    # PSUM writes of successive matmuls land in stream order — so drop it
    # when it would exceed the slot budget.
    for inst in nc.inst_map.values():
        if type(inst).__name__ != "InstMatmult":
            continue
        si = inst.sync_info
        if si is None or len(si.on_wait) <= 1:
            continue
        keep = [w for w in si.on_wait if not w.ant_name.startswith("PE")]
        assert len(keep) == 1, f"unfixable multi-wait matmul: {si.on_wait}"
        si.on_wait = keep
        inst.sync_info = si

    # The kernel-tail drain waits on every sem lane (ACT, PE, DVE + all 8
    # DMAHW lanes) and overflows its wait-slot budget. Every *input* DMA
    # lane is transitively covered by the PE wait (each input DMA has a PE
    # consumer via the real/dummy matmuls above), so only the output DMA's
    # lane plus the engine sems are load-bearing.
    out_lanes = set()
    last_dma = None
    for inst in nc.inst_map.values():
        if type(inst).__name__ == "InstDMACopy":
            last_dma = inst  # output DMA is emitted last
    if last_dma is not None and last_dma.sync_info is not None:
        out_lanes = {u.ant_name for u in last_dma.sync_info.on_update}
    for inst in nc.inst_map.values():
        if type(inst).__name__ != "InstDrain":
            continue
        si = inst.sync_info
        if si is None or len(si.on_wait) <= 1:
            continue
        keep = [w for w in si.on_wait if w.ant_name in out_lanes]
        assert len(keep) <= 1, f"drain still over budget: {[w.ant_name for w in keep]}"
        si.on_wait = keep
        inst.sync_info = si

    _PROGRAM_CACHE[A] = nc
    return nc


def _host_prep(x, xyz, mask, rw1, rw2):
    """Build per-core device inputs. Returns (in_maps, meta)."""
    x = np.ascontiguousarray(x, dtype=np.float32)
    xyz = np.ascontiguousarray(xyz, dtype=np.float32)
    mask = np.asarray(mask)
    rw1 = np.asarray(rw1, dtype=np.float32)
    rw2 = np.asarray(rw2, dtype=np.float32)

    kept = [np.where(mask[z] != 0)[0] for z in range(B)]
    halves = []
    for z in range(B):
        k = kept[z]
        n0 = (len(k) + 1) // 2
        halves.append((k[:n0], k[n0:]))
    max_half = max((max(len(h0), len(h1)) for h0, h1 in halves), default=1)
    A = max(16, -(-max_half // 16) * 16)  # pad to multiple of 16, >=16

    # rw1 strip weights, h padded 100->128 with zeros (cols 0:128 of vcon)
    rw1rows = np.zeros((12, 128), dtype=np.float32)
    for s in range(4):
        rw1rows[3 * s : 3 * s + 3, :H] = rw1

    # W2x[b,h,i] = sum_j rw2[h, i*C+j] x[b,j]; fold 1/sqrt(N)
    rw2r = rw2.reshape(H, C, C)  # [h, i, j]
    in_maps = []
    meta = []
    w2sb_z = {}
    for core in range(N_CORES):
        z, hf = core // 2, core % 2
        a_idx = halves[z][hf]
        n_valid = len(a_idx)
        pad = np.zeros(A, dtype=np.int64)
        pad[:n_valid] = a_idx
        # V3Q: partitions (s,k) rows 3s+k ; cols (b_local*A + a)
        pts = xyz[z]  # [256, 3]
        pa = pts[pad]  # [A, 3]
        vcon = np.empty((12, 128 + 64 * A), dtype=np.float32)
        vcon[:, :128] = rw1rows
        for s in range(4):
            pb = pts[64 * s : 64 * (s + 1)]  # [64, 3]
            d = pb[:, None, :] - pa[None, :, :]
            r = np.sqrt(np.sum(d * d, axis=-1, dtype=np.float32) + 1e-12)  # [64, A]
            for k in range(3):
                bas = np.exp(-GAMMA * (r - CENTERS[k]) ** 2)
                vcon[3 * s + k, 128:] = bas.reshape(-1)
        if z not in w2sb_z:
            w2x = np.tensordot(x[z], rw2r, axes=([1], [2]))  # [b, h, i]
            w2x = np.transpose(w2x, (1, 0, 2)).reshape(H, N * C) / np.sqrt(
                np.float32(N)
            )
            w2sb = np.zeros((128, N * C), dtype=np.float32)
            w2sb[:H] = w2x
            w2sb_z[z] = np.ascontiguousarray(w2sb)
        in_maps.append(
            {
                "vcon": np.ascontiguousarray(vcon),
                "w2sb": w2sb_z[z],
            }
        )
        meta.append((z, hf, n_valid))
    return in_maps, meta, A


def kernel(x, xyz, mask, rw1, rw2, fc3_w, fc3_b, fc2_w, fc2_b):
    global LAST_RESULT
    from concourse.bass_utils import run_bass_kernel_spmd

    in_maps, meta, A = _host_prep(x, xyz, mask, rw1, rw2)
    nc = _build_program(A)
    res = run_bass_kernel_spmd(
        nc,
        in_maps,
        list(range(N_CORES)),
        trace=bool(os.environ.get("BASS_TRACE")),
    )
    LAST_RESULT = res

    pooled = np.zeros((B, C), dtype=np.float64)
    for core in range(N_CORES):
        z, hf, n_valid = meta[core]
        o = res.results[core]["outp"].astype(np.float64)  # [128, A]
        o = o.reshape(4, 32, A).sum(axis=0)  # fold col strips -> [C, A]
        if n_valid:
            pooled[z] += np.abs(o[:, :n_valid]).sum(axis=1)

    mean = pooled.mean(axis=1, keepdims=True)
    std = pooled.std(axis=1, ddof=1, keepdims=True)
    pooled = (pooled - mean) / (std + 1e-6)
    h1 = pooled @ np.asarray(fc3_w, dtype=np.float64) + np.asarray(
        fc3_b, dtype=np.float64
    )
    h1 = np.where(h1 >= 0, h1, LEAKY_SLOPE * h1)
    y = h1 @ np.asarray(fc2_w, dtype=np.float64) + np.asarray(
        fc2_b, dtype=np.float64
    )
    return y.reshape(-1).astype(np.float32)
